# revision 26
# baseline (speedup 1.0000x reference)
"""PixelMixBlock V2 Trainium2 kernel.

Full inputs in, full output out. Data-parallel over the batch: 32 samples
-> 8 NeuronCores x 4 samples. Per sample (on device, all matmuls bf16):

  q = Wq' x          [128, 784]   (Wq' = Wq * 128**-0.25, folds the 1/sqrt(d))
  k = Wq' x_g        [128, 784]   (x_g = x[index], gathered on host)
  v = Wv[:256] x_g   [1, 784]
  sT[c] = k[:,c*112:+112].T @ q            [112, 784] PSUM   (7 chunks)
  E[c]  = exp(sT[c])                       bf16 SBUF
  [num;den] += [v_chunk, 1].T @ E[c]       [2, 784] PSUM accumulate
  m = num/den + (Wv[256]*(1-lam) + bv)     [1, 784]
  O = A @ m.reshape(28,28) @ A.T           (bilinear x8 upsample as matmuls)
  y[1] = 1/(1+exp(-O)); y[0] = 1 - y[1]

exp/sigmoid both use the ACT "exp" table set (sigmoid via exp + DVE
reciprocal) so no activation-table reload ever happens.
"""

import numpy as np
import ml_dtypes

import concourse.bass as bass
import concourse.bacc as bacc
import concourse.mybir as mybir
import concourse.tile as tile
from concourse.bass_utils import run_bass_kernel_spmd

BF16 = mybir.dt.bfloat16
F32 = mybir.dt.float32
AF = mybir.ActivationFunctionType
ALU = mybir.AluOpType

N_CORES = 8
NS = 4            # samples per core
CIN = 256         # input channels
INTER = 128       # Wq output channels
HW = 784          # 28*28
CH = 28
OUT = 224         # 28 * scale_factor(8)
CW = 112          # q-chunk width (784 = 7*112)
NCH = 7           # number of q chunks
NH = 392          # matmul N half (784 = 2*392)
OH = 112          # output row chunk (224 = 2*112)


def _bilinear_matrix(n_in: int, n_out: int) -> np.ndarray:
    # Matches jax.image.resize(method='bilinear') for upscaling:
    # half-pixel sampling with edge clamp.
    scale = n_out / n_in
    A = np.zeros((n_out, n_in), np.float32)
    for i in range(n_out):
        src = (i + 0.5) / scale - 0.5
        j0 = int(np.floor(src))
        f = src - j0
        j0c = min(max(j0, 0), n_in - 1)
        j1c = min(max(j0 + 1, 0), n_in - 1)
        A[i, j0c] += 1.0 - f
        A[i, j1c] += f
    return A


def _build_program(dbg=False):
    nc = bacc.Bacc(
        "TRN2",
        target_bir_lowering=False,
        debug=False,
        enable_asserts=True,
        num_devices=N_CORES,
    )

    xq_d = nc.dram_tensor("xq", [NS, 2, 128, HW], BF16, kind="ExternalInput").ap()
    xk_d = nc.dram_tensor("xk", [NS, 2, 128, HW], BF16, kind="ExternalInput").ap()
    wqt_d = nc.dram_tensor("wqt", [2, 128, INTER], BF16, kind="ExternalInput").ap()
    wvt_d = nc.dram_tensor("wvt", [2, 128, 1], BF16, kind="ExternalInput").ap()
    at_d = nc.dram_tensor("at", [CH, OUT], BF16, kind="ExternalInput").ap()
    cvec_d = nc.dram_tensor("cvec", [1, 1], F32, kind="ExternalInput").ap()
    y_d = nc.dram_tensor("y", [NS, 2, OUT, OUT], F32, kind="ExternalOutput").ap()
    if dbg:
        dq_d = nc.dram_tensor("dq", [NS, 128, HW], BF16, kind="ExternalOutput").ap()
        dk_d = nc.dram_tensor("dk", [NS, 128, HW], BF16, kind="ExternalOutput").ap()
        dv_d = nc.dram_tensor("dv", [NS, 1, HW], BF16, kind="ExternalOutput").ap()
        dw2_d = nc.dram_tensor("dw2", [NS, CW, 2 * NCH], BF16, kind="ExternalOutput").ap()
        de_d = nc.dram_tensor("de", [NS, CW, HW], BF16, kind="ExternalOutput").ap()
        dnd_d = nc.dram_tensor("dnd", [NS, 2, HW], F32, kind="ExternalOutput").ap()
        dmv_d = nc.dram_tensor("dmv", [NS, 1, HW], BF16, kind="ExternalOutput").ap()
        dmh_d = nc.dram_tensor("dmh", [NS, CH, CH], BF16, kind="ExternalOutput").ap()
        du_d = nc.dram_tensor("du", [NS, CH, OUT], BF16, kind="ExternalOutput").ap()

    with tile.TileContext(nc) as tc:
        with (
            tc.tile_pool(name="const", bufs=1) as cpool,
            tc.tile_pool(name="xin", bufs=2) as xpool,
            tc.tile_pool(name="qk", bufs=2) as qkpool,
            tc.tile_pool(name="ew", bufs=4) as epool,
            tc.tile_pool(name="w2p", bufs=2) as w2pool,
            tc.tile_pool(name="vb", bufs=2) as vbpool,
            tc.tile_pool(name="mk", bufs=2) as mkpool,
            tc.tile_pool(name="sg", bufs=3) as sgpool,
            tc.tile_pool(name="scr", bufs=2, space="DRAM") as drpool,
            tc.tile_pool(name="pmm", bufs=2, space="PSUM") as pmm,
            tc.tile_pool(name="pst", bufs=2, space="PSUM") as pst,
            tc.tile_pool(name="pnd", bufs=1, space="PSUM") as pnd,
        ):
            # --- constants ---
            wqt_sb = cpool.tile([128, 2 * INTER], BF16, tag="wqt")
            for c in range(2):
                nc.sync.dma_start(wqt_sb[:, c * INTER:(c + 1) * INTER], wqt_d[c])
            wvt_sb = cpool.tile([128, 2], BF16, tag="wvt")
            for c in range(2):
                nc.sync.dma_start(wvt_sb[:, c:c + 1], wvt_d[c])
            at_sb = cpool.tile([CH, OUT], BF16, tag="at")
            nc.sync.dma_start(at_sb[:], at_d[:])
            cvec_sb = cpool.tile([1, 1], F32, tag="cvec")
            nc.sync.dma_start(cvec_sb[:], cvec_d[:])

            for s in range(NS):
                # --- load x (own) and x_g (gathered) ---
                xq_sb = xpool.tile([128, 2 * HW], BF16, tag="xq")
                xk_sb = xpool.tile([128, 2 * HW], BF16, tag="xk")
                for c in range(2):
                    nc.sync.dma_start(xq_sb[:, c * HW:(c + 1) * HW], xq_d[s, c])
                    nc.sync.dma_start(xk_sb[:, c * HW:(c + 1) * HW], xk_d[s, c])

                # --- q, k: [128, 784] = WqT.T @ x (K=256 in 2 chunks) ---
                q_sb = qkpool.tile([128, HW], BF16, tag="q")
                k_sb = qkpool.tile([128, HW], BF16, tag="k")
                for (src, dst) in ((xq_sb, q_sb), (xk_sb, k_sb)):
                    # [128, 2, 512] so each 392-wide half is PSUM-bank aligned
                    qps = pst.tile([128, 2, 512], F32, tag="st")
                    for h in range(2):
                        for c in range(2):
                            nc.tensor.matmul(
                                qps[:, h, 0:NH],
                                wqt_sb[:, c * INTER:(c + 1) * INTER],
                                src[:, c * HW + h * NH: c * HW + (h + 1) * NH],
                                start=(c == 0),
                                stop=(c == 1),
                            )
                    nc.vector.tensor_copy(
                        dst[:].rearrange("p (h f) -> p h f", h=2),
                        qps[:, :, 0:NH],
                    )

                # --- v: [1, 784] = WvT.T @ x_g ---
                vps = pst.tile([1, 2, 512], F32, tag="st")
                for h in range(2):
                    for c in range(2):
                        nc.tensor.matmul(
                            vps[:, h, 0:NH],
                            wvt_sb[:, c:c + 1],
                            xk_sb[:, c * HW + h * NH: c * HW + (h + 1) * NH],
                            start=(c == 0),
                            stop=(c == 1),
                        )
                # drain v to SBUF (cast bf16), reshape [1, 784] -> [7, 112]
                vsb = vbpool.tile([1, HW], BF16, tag="vsb")
                nc.vector.tensor_copy(
                    vsb[:].rearrange("p (h f) -> p h f", h=2),
                    vps[:, :, 0:NH],
                )
                # w2: [112, 14] bf16; even cols = v chunk, odd cols = 1.0.
                # SBUF APs can't re-split the free axis across partitions, so
                # bounce v through a DRAM scratch to transpose the layout.
                vscr = drpool.tile([1, HW], BF16, tag="vscr")
                nc.sync.dma_start(vscr[:], vsb[:])
                w2 = w2pool.tile([CW, 2 * NCH], BF16, tag="w2")
                nc.vector.memset(w2[:, 1:2 * NCH:2], 1.0)
                nc.sync.dma_start(
                    w2[:, 0:2 * NCH:2],
                    vscr[0, :].rearrange("(a b) -> b a", a=NCH),
                )

                if dbg:
                    nc.sync.dma_start(dq_d[s], q_sb[:])
                    nc.sync.dma_start(dk_d[s], k_sb[:])
                    nc.sync.dma_start(dv_d[s], vsb[:])
                    nc.sync.dma_start(dw2_d[s], w2[:])

                # --- attention chunks: sT -> exp -> num/den accumulate ---
                ndps = pnd.tile([2, 2, 512], F32, tag="nd")
                for c in range(NCH):
                    stps = pst.tile([CW, 2, 512], F32, tag="st")
                    for h in range(2):
                        nc.tensor.matmul(
                            stps[:, h, 0:NH],
                            k_sb[:, c * CW:(c + 1) * CW],
                            q_sb[:, h * NH:(h + 1) * NH],
                            start=True,
                            stop=True,
                        )
                    et = epool.tile([CW, HW], BF16, tag="e")
                    nc.scalar.activation(
                        et[:].rearrange("p (h f) -> p h f", h=2),
                        stps[:, :, 0:NH],
                        AF.Exp,
                    )
                    if dbg and c == 0:
                        nc.sync.dma_start(de_d[s], et[:])
                    for h in range(2):
                        nc.tensor.matmul(
                            ndps[:, h, 0:NH],
                            w2[:, 2 * c:2 * c + 2],
                            et[:, h * NH:(h + 1) * NH],
                            start=(c == 0),
                            stop=(c == NCH - 1),
                            skip_group_check=True,
                        )

                # --- mask vector: m = num/den + const ---
                nd_sb = mkpool.tile([2, HW], F32, tag="nd_sb")
                nc.vector.tensor_copy(
                    nd_sb[:].rearrange("p (h f) -> p h f", h=2),
                    ndps[:, :, 0:NH],
                )
                # den lives on partition 1; DVE ops must start at partition 0,
                # so relocate it with a tiny SBUF->SBUF DMA first.
                den0 = mkpool.tile([1, HW], F32, tag="den0")
                nc.sync.dma_start(den0[:], nd_sb[1:2, :])
                rec = mkpool.tile([1, HW], F32, tag="rec")
                nc.vector.reciprocal(rec[:], den0[:])
                mvec = mkpool.tile([1, HW], BF16, tag="mvec")
                mtmp = mkpool.tile([1, HW], F32, tag="mtmp")
                nc.vector.tensor_tensor(mtmp[:], nd_sb[0:1, :], rec[:], ALU.mult)
                nc.vector.tensor_scalar(
                    mvec[:], mtmp[:], cvec_sb[0:1, 0:1], None, ALU.add
                )
                # reshape to [28, 28] (h on partitions) via DRAM scratch
                mscr = drpool.tile([1, HW], BF16, tag="mscr")
                nc.sync.dma_start(mscr[:], mvec[:])
                mh = mkpool.tile([CH, CH], BF16, tag="mh")
                nc.sync.dma_start(
                    mh[:], mscr[0, :].rearrange("(a b) -> a b", a=CH)
                )
                if dbg:
                    nc.sync.dma_start(dnd_d[s], nd_sb[:])
                    nc.sync.dma_start(dmv_d[s], mvec[:])
                    nc.sync.dma_start(dmh_d[s], mh[:])

                # --- bilinear upsample: U = M.T @ AhT ; O = U_chunk.T @ AwT ---
                ups = pmm.tile([CH, OUT], F32, tag="mm")
                nc.tensor.matmul(ups[:], mh[:], at_sb[:], start=True, stop=True)
                u_sb = mkpool.tile([CH, OUT], BF16, tag="u")
                nc.vector.tensor_copy(u_sb[:], ups[:])
                if dbg:
                    nc.sync.dma_start(du_d[s], u_sb[:])

                for j in range(2):
                    ops = pmm.tile([OH, OUT], F32, tag="mm")
                    nc.tensor.matmul(
                        ops[:], u_sb[:, j * OH:(j + 1) * OH], at_sb[:],
                        start=True, stop=True,
                    )
                    # sigmoid via exp (stays on the exp ACT table set)
                    es = sgpool.tile([OH, OUT], F32, tag="es")
                    nc.scalar.activation(es[:], ops[:], AF.Exp, scale=-1.0)
                    t1 = sgpool.tile([OH, OUT], F32, tag="t1")
                    nc.vector.tensor_scalar(t1[:], es[:], 1.0, None, ALU.add)
                    s1 = sgpool.tile([OH, OUT], F32, tag="s1")
                    nc.vector.reciprocal(s1[:], t1[:])
                    s0 = sgpool.tile([OH, OUT], F32, tag="s0")
                    nc.vector.tensor_scalar(
                        s0[:], s1[:], -1.0, 1.0, ALU.mult, ALU.add
                    )
                    nc.sync.dma_start(y_d[s, 1, j * OH:(j + 1) * OH, :], s1[:])
                    nc.sync.dma_start(y_d[s, 0, j * OH:(j + 1) * OH, :], s0[:])

    nc.compile()
    return nc


_NC_CACHE = {}


def _get_program(dbg=False):
    if dbg not in _NC_CACHE:
        _NC_CACHE[dbg] = _build_program(dbg)
    return _NC_CACHE[dbg]


def kernel(x, lam, index, scale_factor, Wq, bq, Wv, bv):
    x = np.asarray(x, dtype=np.float32)
    lam = np.asarray(lam, dtype=np.float32)
    index = np.asarray(index).astype(np.int64)
    Wq = np.asarray(Wq, dtype=np.float32)
    Wv = np.asarray(Wv, dtype=np.float32)
    bv = np.asarray(bv, dtype=np.float32)

    n, C, h, w = x.shape
    bf = ml_dtypes.bfloat16

    xr = x.reshape(n, C, h * w)
    xg = xr[index]

    s4 = np.float32(INTER) ** np.float32(-0.25)
    WqT = np.ascontiguousarray((Wq * s4).T).astype(bf)          # [256, 128]
    wqt = WqT.reshape(2, 128, INTER)
    WvT = np.ascontiguousarray(Wv[0, :C].reshape(C, 1)).astype(bf)
    wvt = WvT.reshape(2, 128, 1)
    const = np.float32(Wv[0, C] * (1.0 - lam[0]) + bv[0])
    cvec = np.full((1, 1), const, np.float32)
    A = _bilinear_matrix(CH, OUT)
    at = np.ascontiguousarray(A.T).astype(bf)                   # [28, 224]

    xq_all = xr.reshape(n, 2, 128, h * w).astype(bf)
    xk_all = xg.reshape(n, 2, 128, h * w).astype(bf)

    import os
    dbg = bool(os.environ.get("DBGTAPS"))
    nc = _get_program(dbg)
    core_ids = list(range(N_CORES))
    in_maps = []
    for i in core_ids:
        sl = slice(i * NS, (i + 1) * NS)
        in_maps.append({
            "xq": np.ascontiguousarray(xq_all[sl]),
            "xk": np.ascontiguousarray(xk_all[sl]),
            "wqt": wqt,
            "wvt": wvt,
            "at": at,
            "cvec": cvec,
        })

    res = run_bass_kernel_spmd(nc, in_maps, core_ids)
    global LAST_RESULTS
    LAST_RESULTS = res
    out = np.concatenate([r["y"] for r in res.results], axis=0)
    return out.astype(np.float32)


LAST_RESULTS = None


if __name__ == "__main__":
    # smoke test with random data
    rng = np.random.default_rng(0)
    inputs = {
        "x": rng.standard_normal((32, 256, 28, 28), dtype=np.float32),
        "lam": rng.random((1,), dtype=np.float32),
        "index": rng.integers(0, 32, (32,)),
        "scale_factor": 8,
        "Wq": (rng.standard_normal((128, 256)) * 0.01).astype(np.float32),
        "bq": np.zeros((128,), np.float32),
        "Wv": (rng.standard_normal((1, 257)) * 0.01).astype(np.float32),
        "bv": np.zeros((1,), np.float32),
    }
    y = kernel(**inputs)
    print("out", y.shape, y.dtype, float(y.min()), float(y.max()))
